# revision 1
# baseline (speedup 1.0000x reference)
"""BiModal attention kernel for Trainium2 (8 NeuronCores, data-parallel over batch).

Per core (one batch b): x, y: [2048, 128] fp32.
  S = x @ y.T                    (float32r matmuls, [2048, 2048])
  E = exp(S)                     (unshifted; softmax is shift-invariant and
                                  |S| <~ 67 so exp stays in fp32/bf16 range)
  a1 = (E @ y) / rowsum(E) * x
  a2 = (E.T @ x) / colsum(E) * y
  out = concat([a1, a2], -1)     ([2048, 256])

Layout: rows are relabeled r = 16*p + b (p = SBUF partition, b = block index)
so every DRAM transfer is contiguous per partition; the relabeling is applied
consistently to s and t everywhere, so the math is unchanged.

x^T / y^T (d-major, needed as f32r matmul operands) are built without the
tensor engine: split into bf16 hi/lo pairs (DVE/ACT), transpose both with the
DMA xbar, and re-merge hi+lo into f32r on GpSimd. bf16(hi)+bf16(lo) carries
~16 mantissa bits >= f32r's ~12, so S keeps f32r accuracy.

Main loop: two 1024-wide column panels; per (row block i): S matmuls (f32r)
-> exp (ACT, PSUM->SBUF bf16, fused row-sum accum) -> xbar transpose of E
into ET -> DVE column-sum partials. o1T chunks (contract over t) interleave
one panel behind to keep PE dense; o2T chunks + o1T tail + epilogues
(PE retranspose + fused gate on DVE) finish.
"""
import sys

sys.path.insert(0, "/opt/trn_rl_repo")

import os
import numpy as np

import concourse.bass as bass
import concourse.mybir as mybir
import concourse.tile as tile
from concourse.tile_rust import add_dep_helper
from concourse import bacc
from concourse.bass_utils import run_bass_kernel_spmd
from concourse.masks import make_identity

f32 = mybir.dt.float32
f32r = mybir.dt.float32r
bf16 = mybir.dt.bfloat16

B = 8
S = 2048
D = 128
P = 128
NB = S // P          # 16 blocks
NP = 2               # panels
PW = S // NP         # panel width (1024)
PB = PW // P         # blocks per panel (8)

_NC_CACHE = None
LAST_EXEC_NS = None


def _build_program(nc):
    x_d = nc.dram_tensor("x", [S, D], f32, kind="ExternalInput").ap()
    y_d = nc.dram_tensor("y", [S, D], f32, kind="ExternalInput").ap()
    out_d = nc.dram_tensor("out", [S, 2 * D], f32, kind="ExternalOutput").ap()

    # contiguous-per-partition views; row r = 16*p + b
    x_dv = x_d.rearrange("(p b) d -> p b d", p=P)      # [128, 16, 128]
    y_dv = y_d.rearrange("(p b) d -> p b d", p=P)
    out_dv = out_d.rearrange("(p b) c -> p b c", p=P)  # [128, 16, 256]

    Exp = mybir.ActivationFunctionType.Exp
    MUL = mybir.AluOpType.mult
    ADD = mybir.AluOpType.add
    SUBR = mybir.AluOpType.subtract
    AX = mybir.AxisListType.X

    with tile.TileContext(nc) as tc:
        with (
            tc.tile_pool(name="sb", bufs=1) as sb,
            tc.tile_pool(name="tp2", bufs=2) as tp2,
            tc.tile_pool(name="tpf", bufs=2) as tpf,
            tc.tile_pool(name="stg", bufs=6) as stg,
            tc.tile_pool(name="ps", bufs=1, space="PSUM") as ps,
        ):
            # ---- persistent SBUF tensors ----
            y_sb = tpf.tile([P, NB, D], f32, tag="vf32")
            x_sb = tpf.tile([P, NB, D], f32, tag="vf32")
            x_hi = sb.tile([P, NB, D], bf16, tag="x_hi")   # doubles as bf16 x
            y_hi = sb.tile([P, NB, D], bf16, tag="y_hi")   # doubles as bf16 y
            x_lo = sb.tile([P, NB, D], bf16, tag="x_lo")
            y_lo = sb.tile([P, NB, D], bf16, tag="y_lo")
            xT = sb.tile([P, NB, P], f32r, tag="xT")       # [d, sb, sp]
            yT = sb.tile([P, NB, P], f32r, tag="yT")       # [d, tb, tp]
            E = sb.tile([P, NB, S], bf16, tag="E")         # [sp, sb, t-pos]
            ET = sb.tile([P, NB, S], bf16, tag="ET")       # [tp, tb, s-pos]
            oT_pool_a = sb.tile([P, S], f32, tag="oT", name="oT_a")
            o1T_sb = oT_pool_a                             # [d, s-pos]
            ident = sb.tile([P, P], f32, tag="ident")
            l1p = sb.tile([P, 2 * NB], f32, tag="l1p")     # [sp, 2*i+ct]
            l2p = sb.tile([P, NB, NB], f32, tag="l2p")     # [tp, tb, i]
            l1 = sb.tile([P, NB], f32, tag="l1")
            l2 = sb.tile([P, NB], f32, tag="l2")
            r1 = sb.tile([P, NB], f32, tag="r1")
            r2 = sb.tile([P, NB], f32, tag="r2")

            make_identity(nc, ident[:])
            nc.sync.dma_start(y_sb[:, 0:PB], y_dv[:, 0:PB])
            nc.sync.dma_start(x_sb[:], x_dv)
            nc.sync.dma_start(y_sb[:, PB:NB], y_dv[:, PB:NB])

            # ---- prologue: xT/yT via hi/lo bf16 split + xbar transpose ----
            # hi = bf16(v) on ACT; lo = bf16(v - hi) on DVE;
            # xbar: [p, (b d)] -> [d, b, p]; merge hi+lo -> f32r on GpSimd.
            def build_T(v_sb, v_hi, v_lo, vT, name, halves=(0, 1)):
                tT_hi = tp2.tile([P, NB, P], bf16, tag="tT_hi", name=f"th_{name}")
                tT_lo = tp2.tile([P, NB, P], bf16, tag="tT_lo", name=f"tl_{name}")
                for h in halves:
                    sl = slice(h * PB, (h + 1) * PB)
                    nc.scalar.copy(v_hi[:, sl], v_sb[:, sl])
                    nc.vector.tensor_tensor(v_lo[:, sl], v_sb[:, sl],
                                            v_hi[:, sl], op=SUBR)
                    nc.sync.dma_start_transpose(
                        tT_hi[:, sl, :], v_hi[:, sl].rearrange("p b d -> p (b d)"))
                    nc.sync.dma_start_transpose(
                        tT_lo[:, sl, :], v_lo[:, sl].rearrange("p b d -> p (b d)"))
                    # first merge on DVE (fast, idle at head), rest on GpSimd
                    eng = nc.vector if (h == halves[0]) else nc.gpsimd
                    m = eng.tensor_tensor(vT[:, sl, :], tT_hi[:, sl, :],
                                          tT_lo[:, sl, :], op=ADD)
                return m

            m_y0 = build_T(y_sb, y_hi, y_lo, yT, "y", halves=(0,))
            build_T(x_sb, x_hi, x_lo, xT, "x")
            build_T(y_sb, y_hi, y_lo, yT, "y2", halves=(1,))

            # ---- main: panels of 1024 columns ----
            s_rot_a = ps.tile([P, PW], f32, tag="A0", name="s_rot_a")
            s_rot_b = ps.tile([P, PW], f32, tag="A1", name="s_rot_b")
            s_rot = [s_rot_a, s_rot_b]                   # S psum, 2-deep rotation
            o1_ps = ps.tile([P, 4, 512], f32, tag="B")   # o1T accumulator

            def o1_chunk(tb, pin=None):
                for q in range(4):
                    mm = nc.tensor.matmul(o1_ps[:, q, :], y_hi[:, tb, :],
                                          ET[:, tb, q * 512:(q + 1) * 512],
                                          start=(tb == 0), stop=(tb == NB - 1))
                    if q == 0 and pin is not None:
                        add_dep_helper(mm.ins, pin.ins, sync=False,
                                       reason="keep chunk at its emission slot")

            # PE warm-up: dense dummy matmuls so HAM unthrottles before S(0);
            # gated on the first merged data so they run during the x prologue
            yh_f = y_hi[:].rearrange("p b d -> p (b d)")
            for w in range(16):
                wm = nc.tensor.matmul(s_rot[0][:, 0:256], y_hi[:, 0, :],
                                      yh_f[:, 0:256], start=True, stop=True)
                if w == 0:
                    add_dep_helper(wm.ins, m_y0.ins, sync=True,
                                   reason="warmup starts once merges begin")

            yT_f = yT[:].rearrange("p b d -> p (b d)")
            for ct in range(NP):
                c0 = ct * PW
                for i in range(NB):
                    xti = xT[:, i, :]
                    slot = s_rot[i % 2][:]
                    nc.tensor.matmul(slot[:, 0:512], xti,
                                     yT_f[:, c0:c0 + 512], start=True, stop=True)
                    sm = nc.tensor.matmul(slot[:, 512:1024], xti,
                                          yT_f[:, c0 + 512:c0 + 1024],
                                          start=True, stop=True)
                    # interleave one o1T chunk of the previous panel (lagged so
                    # the chunk's transposed inputs are ready when PE reaches it)
                    if ct > 0 and 3 <= i < 3 + PB:
                        o1_chunk((ct - 1) * PB + (i - 3), pin=sm)
                    nc.scalar.activation(E[:, i, c0:c0 + PW], slot, Exp,
                                         accum_out=l1p[:, 2 * i + ct:2 * i + ct + 1])
                    nc.sync.dma_start_transpose(
                        ET[:, ct * PB:(ct + 1) * PB, i * P:(i + 1) * P],
                        E[:, i, c0:c0 + PW])
                    nc.vector.tensor_reduce(
                        l2p[:, ct * PB:(ct + 1) * PB, i],
                        ET[:, ct * PB:(ct + 1) * PB, i * P:(i + 1) * P],
                        axis=AX, op=ADD)

            # ---- normalizers ----
            nc.vector.tensor_reduce(l1[:], l1p[:].rearrange("p (i c) -> p i c", c=2),
                                    axis=AX, op=ADD)
            nc.vector.reciprocal(r1[:], l1[:])
            nc.vector.tensor_reduce(l2[:], l2p[:], axis=AX, op=ADD)
            nc.vector.reciprocal(r2[:], l2[:])

            # ---- final phase: o2T + trailing o1T chunks + epilogue 1 ----
            x_sb2 = tpf.tile([P, NB, D], f32, tag="vf32")
            nc.sync.dma_start(x_sb2[:], x_dv)
            y_sb2 = tpf.tile([P, NB, D], f32, tag="vf32")
            nc.sync.dma_start(y_sb2[:], y_dv)

            o2_ps_a = ps.tile([P, 2, 512], f32, tag="A0")
            o2_ps_b = ps.tile([P, 2, 512], f32, tag="A1")
            o2_q = [o2_ps_a[:, 0, :], o2_ps_a[:, 1, :], o2_ps_b[:, 0, :], o2_ps_b[:, 1, :]]

            e1_ps = None

            def epi1_step(j, pin=None):
                st1 = stg.tile([P, D], f32, tag="st", name=f"st1_{j}")
                tr = nc.tensor.transpose(e1_ps[:, j % 4, 0:P],
                                         o1T_sb[:, j * P:(j + 1) * P], ident[:])
                if pin is not None:
                    add_dep_helper(tr.ins, pin.ins, sync=False,
                                   reason="keep epi1 at its emission slot")
                nc.vector.scalar_tensor_tensor(st1[:], e1_ps[:, j % 4, 0:P],
                                               r1[:, j:j + 1], x_sb2[:, j, :],
                                               op0=MUL, op1=MUL)
                nc.sync.dma_start(out_dv[:, j, 0:D], st1[:])

            LAG = 4
            for i in range(NB):
                for q in range(4):
                    om = nc.tensor.matmul(o2_q[q], x_hi[:, i, :],
                                          E[:, i, q * 512:(q + 1) * 512],
                                          start=(i == 0), stop=(i == NB - 1))
                if LAG <= i < LAG + PB:
                    o1_chunk((NP - 1) * PB + (i - LAG), pin=om)
                if i == LAG + PB - 1:
                    # all o1T chunks issued; drain accumulator and start epi-1
                    nc.scalar.copy(o1T_sb[:, 0:1024],
                                   o1_ps[:, 0:2].rearrange("p a b -> p (a b)"))
                    nc.scalar.copy(o1T_sb[:, 1024:2048],
                                   o1_ps[:, 2:4].rearrange("p a b -> p (a b)"))
                    e1_ps = ps.tile([P, 4, 512], f32, tag="B")
                if i >= LAG + PB:
                    for k in range(4):
                        epi1_step(4 * (i - LAG - PB) + k, pin=om if k == 0 else None)

            o2T_sb = sb.tile([P, S], f32, tag="oT", name="oT_b")
            nc.scalar.copy(o2T_sb[:, 0:1024], o2_ps_a[:].rearrange("p a b -> p (a b)"))
            nc.scalar.copy(o2T_sb[:, 1024:2048], o2_ps_b[:].rearrange("p a b -> p (a b)"))

            # ---- epilogue 2: a2 = o2 * y * r2 (staged into dead E space) ----
            e2_rot = [ps.tile([P, 512], f32, tag="A0", name="e2a"),
                      ps.tile([P, 512], f32, tag="A1", name="e2b")]
            for j in range(NB):
                st2 = stg.tile([P, D], f32, tag="st", name=f"st2_{j}")
                e2t = e2_rot[j % 2]
                nc.tensor.transpose(e2t[:, 0:P],
                                    o2T_sb[:, j * P:(j + 1) * P], ident[:])
                nc.vector.scalar_tensor_tensor(st2[:], e2t[:, 0:P],
                                               r2[:, j:j + 1], y_sb2[:, j, :],
                                               op0=MUL, op1=MUL)
                nc.sync.dma_start(out_dv[:, j, D:2 * D], st2[:])

    nc.compile()
    return nc


def _get_nc():
    global _NC_CACHE
    if _NC_CACHE is None:
        nc = bacc.Bacc("TRN2", target_bir_lowering=False, debug=False,
                       num_devices=B)
        _NC_CACHE = _build_program(nc)
    return _NC_CACHE


def kernel(x, y):
    global LAST_EXEC_NS
    nc = _get_nc()
    x = np.asarray(x, dtype=np.float32)
    y = np.asarray(y, dtype=np.float32)
    in_maps = [
        {"x": np.ascontiguousarray(x[b]), "y": np.ascontiguousarray(y[b])}
        for b in range(B)
    ]
    trace = bool(int(os.environ.get("KERNEL_TRACE", "0")))
    res = run_bass_kernel_spmd(nc, in_maps, list(range(B)), trace=trace)
    LAST_EXEC_NS = res.exec_time_ns
    return np.stack([res.results[b]["out"] for b in range(B)], axis=0)



# revision 8
# speedup vs baseline: 1.1465x; 1.1465x over previous
"""BiModal attention kernel for Trainium2 (8 NeuronCores, data-parallel over batch).

Per core (one batch b): x, y: [2048, 128] fp32.
  S = x @ y.T                    (f32r matmuls, [2048, 2048])
  E = exp(S)                     (unshifted; |S| <~ 67 so exp stays in range)
  a1 = (E @ y) / rowsum(E) * x
  a2 = (E.T @ x) / colsum(E) * y
  out = concat([a1, a2], -1)     ([2048, 256])

Rows are relabeled r = 16*p + b (p = SBUF partition, b = block index) so DRAM
transfers are contiguous per partition; applied consistently to s and t.
Score columns are enumerated c = tb*128 + tp (t-row = 16*tp + tb), matching
the yT streaming order.

v2 (row-major) structure:
  - xT/yT built by PE transposes from the f32 loads (exact, no hi/lo DMA).
  - one iteration per (row i, panel ct): S (2 f32r matmuls) -> exp -> o2
    chunk matmuls for the previous half-row -> lagged o1 chunk matmuls.
  - after both panels of row i: ONE xbar transpose E[:,i,:] -> ET[:,i,:,:]
    ([128,2048], 4KB contiguous on both sides, dst groups uniform 256B).
  - col-sum partials (l2p) per row on DVE in bf16 (2x mode), f32 fold.
  - o1 accumulates in 2 PSUM banks (q = s-quarters, two banks ping-pong,
    drained to bf16 o1t as each quarter completes); o2 in 4 banks.
  - o1/o2 return to [s-part, d] via full-width xbar transposes; gating on
    DVE; 4 half-MB stores on the ACT HWDGE ring.
"""
import sys

sys.path.insert(0, "/opt/trn_rl_repo")

import os
import numpy as np

import concourse.bass as bass
import concourse.mybir as mybir
import concourse.tile as tile
from concourse.tile_rust import add_dep_helper
from concourse import bacc
from concourse.bass_utils import run_bass_kernel_spmd
from concourse.masks import make_identity

f32 = mybir.dt.float32
f32r = mybir.dt.float32r
bf16 = mybir.dt.bfloat16

B = 8
S = 2048
D = 128
P = 128
NB = S // P          # 16 row blocks
NP = 2               # panels
PW = S // NP         # panel width (1024)
PB = PW // P         # t-blocks per panel (8)

_NC_CACHE = None
LAST_EXEC_NS = None

DEBUG = bool(int(os.environ.get("KERNEL_DEBUG", "0")))


def _build_program(nc):
    x_d = nc.dram_tensor("x", [S, D], f32, kind="ExternalInput").ap()
    y_d = nc.dram_tensor("y", [S, D], f32, kind="ExternalInput").ap()
    out_d = nc.dram_tensor("out", [S, 2 * D], f32, kind="ExternalOutput").ap()

    x_dv = x_d.rearrange("(p b) d -> p b d", p=P)      # [128, 16, 128]
    y_dv = y_d.rearrange("(p b) d -> p b d", p=P)
    out_dv = out_d.rearrange("(p b) c -> p b c", p=P)  # [128, 16, 256]

    Exp = mybir.ActivationFunctionType.Exp
    MUL = mybir.AluOpType.mult
    ADD = mybir.AluOpType.add
    AX = mybir.AxisListType.X

    with tile.TileContext(nc) as tc:
        with (
            tc.tile_pool(name="sb", bufs=1) as sb,
            tc.tile_pool(name="shp", bufs=1) as shp,
            tc.tile_pool(name="shq", bufs=1) as shq,
            tc.tile_pool(name="ps", bufs=1, space="PSUM") as ps,
        ):
            # ---- persistent SBUF ----
            x_sb = shp.tile([P, NB, D], f32, tag="P", name="x_f32")
            y_sb = shq.tile([P, NB, D], f32, tag="Q", name="y_f32")
            xT = sb.tile([P, NB, P], f32r, tag="xT")       # [d, i, sp]
            yT = sb.tile([P, NB, P], f32r, tag="yT")       # [d, tb, tp]
            x_bf = sb.tile([P, NB, D], bf16, tag="x_bf")
            y_bf = sb.tile([P, NB, D], bf16, tag="y_bf")
            E = sb.tile([P, NB, S], bf16, tag="E")         # [sp, i, c]
            ET = sb.tile([P, NB, NB, P], bf16, tag="ET")   # [tp, i, tb, sp]
            o1t = sb.tile([P, S], bf16, tag="o1t")         # [d, s]
            o2t = sb.tile([P, S], bf16, tag="o2t")         # [d, c]
            o1s = sb.tile([P, NB, D], bf16, tag="o1s")     # [sp, i, d]
            o2s = sb.tile([P, NB, D], bf16, tag="o2s")     # [tp, tb, d]
            ident = sb.tile([P, P], f32, tag="ident")
            warm = sb.tile([P, 2, P], bf16, tag="warm")
            l1p = sb.tile([P, 2 * NB], f32, tag="l1p")     # col = 2*i+ct
            l2p = sb.tile([P, NB, NB], bf16, tag="l2p")    # [tp, i, tb]
            l1 = sb.tile([P, NB], f32, tag="l1")
            l2 = sb.tile([P, NB], f32, tag="l2")
            r1 = sb.tile([P, NB], f32, tag="r1")
            r2 = sb.tile([P, NB], f32, tag="r2")

            # ---- PSUM (8 banks exactly) ----
            s_ps = ps.tile([P, PW], f32, tag="A")          # 2 banks
            o1_ps = ps.tile([P, 2, 512], f32, tag="B")     # 2 banks
            o2_ps = ps.tile([P, 4, 512], f32, tag="C")     # 4 banks
            pTs = [ps.tile([P, 4, P], f32, tag="B", name="pT0"),
                   ps.tile([P, 4, P], f32, tag="C", name="pT1")]

            last_pe = [None]

            def pe_chain(mm):
                if last_pe[0] is not None:
                    add_dep_helper(mm.ins, last_pe[0].ins, sync=False,
                                   reason="keep PE emission order")
                last_pe[0] = mm
                return mm

            # ---- warmup: unthrottle HAM during the loads ----
            make_identity(nc, ident[:])
            nc.gpsimd.memset(warm[:], 0)
            wf = warm[:].rearrange("p a b -> p (a b)")     # [128, 256]
            for w in range(14):
                pe_chain(nc.tensor.matmul(s_ps[:, 0:256], warm[:, 0, :], wf,
                                          start=True, stop=True))

            # ---- loads (sync ring) ----
            nc.sync.dma_start(y_sb[:, 0:PB], y_dv[:, 0:PB])
            nc.sync.dma_start(x_sb[:, 0:PB], x_dv[:, 0:PB])
            nc.sync.dma_start(y_sb[:, PB:NB], y_dv[:, PB:NB])
            nc.sync.dma_start(x_sb[:, PB:NB], x_dv[:, PB:NB])

            # ---- prologue: exact xT/yT via PE transposes ----
            pp = [0]

            def prologue_T(v_sb, vT, v_bf, half):
                for k in range(2):
                    pT = pTs[pp[0] % 2]
                    pp[0] += 1
                    b0 = half * PB + k * 4
                    for b4 in range(4):
                        pe_chain(nc.tensor.transpose(
                            pT[:, b4, :], v_sb[:, b0 + b4, :], ident[:]))
                    nc.vector.tensor_copy(vT[:, b0:b0 + 4, :], pT[:])
                sl = slice(half * PB, (half + 1) * PB)
                nc.scalar.copy(v_bf[:, sl], v_sb[:, sl])

            prologue_T(y_sb, yT, y_bf, 0)
            prologue_T(x_sb, xT, x_bf, 0)
            prologue_T(y_sb, yT, y_bf, 1)
            prologue_T(x_sb, xT, x_bf, 1)

            # ---- o1 chunk schedule ----
            # chunk (tb, q) = one N=512 matmul over ET[:, 4q:4q+4, tb, :];
            # needs T(4q+3), emitted at git 2*(4q+3)+1 -> ready 8q+9.
            NG = 2 * NB
            pops = [[] for _ in range(NG + 1)]
            queue = []
            for q in range(4):
                for tb in range(NB):
                    queue.append((8 * q + 9, tb, q))
            queue.sort(key=lambda t: t[0])
            qi = 0
            for g in range(NG + 1):
                cap = 3 if g < NG else 64
                while qi < len(queue) and (queue[qi][0] <= g or g == NG) \
                        and cap > 0:
                    pops[g].append(queue[qi][1:])
                    qi += 1
                    cap -= 1
            assert qi == len(queue)

            o1_count = [0] * 4

            def emit_o1(tb, q):
                c = o1_count[q]
                o1_count[q] += 1
                pe_chain(nc.tensor.matmul(
                    o1_ps[:, q % 2, :], y_bf[:, tb, :],
                    ET[:, 4 * q:4 * q + 4, tb, :],
                    start=(c == 0), stop=(c == NB - 1)))
                if o1_count[q] == NB:
                    nc.vector.tensor_copy(o1t[:, q * 512:(q + 1) * 512],
                                          o1_ps[:, q % 2, :])

            def emit_o2_for(prev_git):
                pi, pct = divmod(prev_git, 2)
                for qq in (2 * pct, 2 * pct + 1):
                    pe_chain(nc.tensor.matmul(
                        o2_ps[:, qq, :], x_bf[:, pi, :],
                        E[:, pi, qq * 512:(qq + 1) * 512],
                        start=(pi == 0), stop=(pi == NB - 1)))

            # ---- main: 16 rows x 2 panels ----
            yTf = yT[:].rearrange("p b d -> p (b d)")      # [128, 2048]
            for i in range(NB):
                for ct in range(NP):
                    git = 2 * i + ct
                    c0 = ct * PW
                    pe_chain(nc.tensor.matmul(
                        s_ps[:, 0:512], xT[:, i, :], yTf[:, c0:c0 + 512],
                        start=True, stop=True))
                    pe_chain(nc.tensor.matmul(
                        s_ps[:, 512:1024], xT[:, i, :],
                        yTf[:, c0 + 512:c0 + 1024], start=True, stop=True))
                    nc.scalar.activation(E[:, i, c0:c0 + PW], s_ps[:], Exp,
                                         accum_out=l1p[:, git:git + 1])
                    if git >= 1:
                        emit_o2_for(git - 1)
                    for (tb, q) in pops[git]:
                        emit_o1(tb, q)
                    if ct == 1:
                        nc.sync.dma_start_transpose(ET[:, i, :, :],
                                                    E[:, i, :])
                        with nc.allow_low_precision("l2 partials in bf16"):
                            nc.vector.tensor_reduce(
                                l2p[:, i, :], ET[:, i, :, :], axis=AX, op=ADD)

            # ---- tail ----
            emit_o2_for(NG - 1)
            for (tb, q) in pops[NG]:
                emit_o1(tb, q)

            nc.vector.tensor_reduce(
                l1[:], l1p[:].rearrange("p (a b) -> p a b", a=NB), axis=AX,
                op=ADD)
            nc.vector.reciprocal(r1[:], l1[:])
            nc.vector.tensor_reduce(
                l2[:], l2p[:].rearrange("p a b -> p b a"), axis=AX, op=ADD)
            nc.vector.reciprocal(r2[:], l2[:])

            nc.vector.tensor_copy(o2t[:, 0:PW],
                                  o2_ps[:, 0:2].rearrange("p a b -> p (a b)"))
            nc.scalar.copy(o2t[:, PW:S],
                           o2_ps[:, 2:4].rearrange("p a b -> p (a b)"))
            nc.sync.dma_start_transpose(o2s[:, :, :], o2t[:])
            nc.sync.dma_start_transpose(o1s[:, :, :], o1t[:])

            outP = shp.tile([P, NB, D], f32, tag="P", name="outP")
            outQ = shq.tile([P, NB, D], f32, tag="Q", name="outQ")
            for b in range(NB):
                nc.vector.scalar_tensor_tensor(
                    outQ[:, b, :], o2s[:, b, :], r2[:, b:b + 1], y_bf[:, b, :],
                    op0=MUL, op1=MUL)
                if b == PB - 1:
                    nc.scalar.dma_start(out_dv[:, 0:PB, D:2 * D],
                                        outQ[:, 0:PB, :])
            nc.scalar.dma_start(out_dv[:, PB:NB, D:2 * D], outQ[:, PB:NB, :])
            for b in range(NB):
                nc.vector.scalar_tensor_tensor(
                    outP[:, b, :], o1s[:, b, :], r1[:, b:b + 1], x_bf[:, b, :],
                    op0=MUL, op1=MUL)
                if b == PB - 1:
                    nc.scalar.dma_start(out_dv[:, 0:PB, 0:D], outP[:, 0:PB, :])
            nc.scalar.dma_start(out_dv[:, PB:NB, 0:D], outP[:, PB:NB, :])

            if DEBUG:
                dbg_specs = [
                    ("dbg_xT", xT, [P, NB, P], f32, True),
                    ("dbg_yT", yT, [P, NB, P], f32, True),
                    ("dbg_E", E, [P, NB, S], bf16, False),
                    ("dbg_ET", ET, [P, NB, NB, P], bf16, False),
                    ("dbg_o1t", o1t, [P, S], bf16, False),
                    ("dbg_o2t", o2t, [P, S], bf16, False),
                    ("dbg_o1s", o1s, [P, NB, D], bf16, False),
                    ("dbg_o2s", o2s, [P, NB, D], bf16, False),
                    ("dbg_l1p", l1p, [P, 2 * NB], f32, False),
                    ("dbg_l2p", l2p, [P, NB, NB], bf16, False),
                    ("dbg_l1", l1, [P, NB], f32, False),
                    ("dbg_l2", l2, [P, NB], f32, False),
                ]
                for name, t, shp_, dt_, cast_ in dbg_specs:
                    dd = nc.dram_tensor(name, shp_, dt_,
                                        kind="ExternalOutput").ap()
                    src = t[:].bitcast(f32) if cast_ else t[:]
                    nc.sync.dma_start(dd, src)

    nc.compile()
    return nc


def _get_nc():
    global _NC_CACHE
    if _NC_CACHE is None:
        nc = bacc.Bacc("TRN2", target_bir_lowering=False, debug=False,
                       num_devices=B)
        _NC_CACHE = _build_program(nc)
    return _NC_CACHE


def kernel(x, y):
    global LAST_EXEC_NS
    nc = _get_nc()
    x = np.asarray(x, dtype=np.float32)
    y = np.asarray(y, dtype=np.float32)
    in_maps = [
        {"x": np.ascontiguousarray(x[b]), "y": np.ascontiguousarray(y[b])}
        for b in range(B)
    ]
    trace = bool(int(os.environ.get("KERNEL_TRACE", "0")))
    res = run_bass_kernel_spmd(nc, in_maps, list(range(B)), trace=trace)
    LAST_EXEC_NS = res.exec_time_ns
    return np.stack([res.results[b]["out"] for b in range(B)], axis=0)
